# revision 13
# baseline (speedup 1.0000x reference)
"""Trainium2 Bass kernel for CSDNet (cascaded SH-fit + 3D conv refinement).

Math reformulation (host-side, exact up to fp assoc):
  AQ_Tb = AQ^T b per voxel;  A = AQ^T AQ + dr*I;  A18 = AQ_s^T AQ_s + 0.01 I
  c0 = M0 @ AQ_Tb  with  M0[IDX,IDX] = inv(A18)      (low-order init)
  u  = invA @ AQ_Tb                                   (data-consistency term)
  c_{i+1} = u + (dr*invA) @ g_i,   g_i = gate(cascade_i(c_i)) (+ c_i residual)
Both u and c0 come from one fused projection  [u; 0pad; c0] = G @ b_vox with
G = [invA; 0; M0] @ AQ^T  (111 x 300), so the big b read happens once.  The
projection runs as bf16 hi+lo split matmuls (~fp32 accuracy); u stays
resident in SBUF (fp32), c roundtrips DRAM as bf16.

Sharding: D axis split 8 ways; each core gets a 13-row overlapping slab
(5 output rows + 4 halo rows each side) so no inter-core communication is
needed.  Valid 3x3x3 convs are shifted-matmul accumulation over flattened
voxels; x-adjacent tap pairs share one K=94 matmul via a 1-shifted copy
of the activation rows (18 passes/conv instead of 27).  Compute is
restricted to the shrinking valid-H window per cascade.  Heavy DMA goes
through SWDGE (gpsimd), which spreads across all 16 SDMA engines; the
HWDGE rings (pinned to one engine here) carry only the cbuf row loads.
"""

import numpy as np
import ml_dtypes

import concourse.bacc as bacc
import concourse.bass as bass
import concourse.mybir as mybir
import concourse.tile as tile
from concourse.bass_utils import run_bass_kernel_spmd

F32 = mybir.dt.float32
BF16 = mybir.dt.bfloat16
BF = ml_dtypes.bfloat16

# problem constants (hardcoded per contract)
S = 300
K = 47
HID = 96
NCASC = 4
DFULL = 48
W = 48
HW = 48 * 48
NCORES = 8
SD = 13
NSLAB = SD * HW
OUTD = 5
R = 4                   # circular row slots
PAD = 49
NT_A = 512
IDX = np.r_[0:16, 45:47]


def _split_cols(width):
    out = []
    rem = width
    while rem > 0:
        if rem > 512:
            nt = 512 if rem - 512 >= 256 else rem - 256
        else:
            nt = rem
        out.append(nt)
        rem -= nt
    return out


def build_program():
    nc = bacc.Bacc(target_bir_lowering=False, trn_type="TRN2")

    bcat = nc.dram_tensor("bcat", [59, 100, 6, 512], BF16, kind="ExternalInput")
    GT = nc.dram_tensor("GT", [S, 111], BF16, kind="ExternalInput")
    GTl = nc.dram_tensor("GTl", [S, 111], BF16, kind="ExternalInput")
    wp = nc.dram_tensor("wp", [NCASC, 9, 94, HID], BF16, kind="ExternalInput")
    ws = nc.dram_tensor("ws", [NCASC, 9, K, HID], BF16, kind="ExternalInput")
    w2t = nc.dram_tensor("w2t", [NCASC, HID, HID], BF16, kind="ExternalInput")
    w3t = nc.dram_tensor("w3t", [NCASC, HID, 94], BF16, kind="ExternalInput")
    sT = nc.dram_tensor("sT", [K, K], BF16, kind="ExternalInput")
    b12 = nc.dram_tensor("b12", [HID, NCASC, 2], F32, kind="ExternalInput")
    b3lo = nc.dram_tensor("b3lo", [K, NCASC], F32, kind="ExternalInput")
    b3hi = nc.dram_tensor("b3hi", [K, NCASC], F32, kind="ExternalInput")
    zz = nc.dram_tensor("zz", [94, 64], BF16, kind="ExternalInput")

    c_a = nc.dram_tensor("c_a", [K, NSLAB + 64], BF16, kind="Internal")
    c_b = nc.dram_tensor("c_b", [K, NSLAB + 64], BF16, kind="Internal")
    cout = nc.dram_tensor("cout", [K, OUTD, HW], F32, kind="ExternalOutput")

    AF = mybir.ActivationFunctionType
    ALU = mybir.AluOpType

    with tile.TileContext(nc) as tc:
        with (
            tc.tile_pool(name="consts", bufs=1) as cp,
            tc.tile_pool(name="ubig", bufs=1) as up,
            tc.tile_pool(name="bstream", bufs=2) as bp,
            tc.tile_pool(name="evict", bufs=2) as ep,
            tc.tile_pool(name="cbufp", bufs=1) as cbp,
            tc.tile_pool(name="rows", bufs=2) as rp,
            tc.tile_pool(name="acts", bufs=3) as ap_,
            tc.tile_pool(name="pp", bufs=1, space="PSUM") as pp,
        ):
            # ---- constants ----
            G_sb = cp.tile([100, 3, 111], BF16)
            nc.sync.dma_start(
                out=G_sb[:], in_=GT[:].rearrange("(c p) m -> p c m", p=100)
            )
            Gl_sb = cp.tile([100, 3, 111], BF16)
            nc.sync.dma_start(
                out=Gl_sb[:], in_=GTl[:].rearrange("(c p) m -> p c m", p=100)
            )
            wp_sb = cp.tile([94, NCASC, 9, HID], BF16)
            nc.sync.dma_start(out=wp_sb[:], in_=wp[:].rearrange("i t p m -> p i t m"))
            ws_sb = cp.tile([K, NCASC, 9, HID], BF16)
            nc.sync.dma_start(out=ws_sb[:], in_=ws[:].rearrange("i t p m -> p i t m"))
            w2_sb = cp.tile([HID, NCASC, HID], BF16)
            nc.sync.dma_start(out=w2_sb[:], in_=w2t[:].rearrange("i p m -> p i m"))
            w3_sb = cp.tile([HID, NCASC, 94], BF16)
            nc.sync.dma_start(out=w3_sb[:], in_=w3t[:].rearrange("i p m -> p i m"))
            s_sb = cp.tile([K, K], BF16)
            nc.sync.dma_start(out=s_sb[:], in_=sT[:])
            b12_sb = cp.tile([HID, NCASC, 2], F32)
            nc.sync.dma_start(out=b12_sb[:], in_=b12[:])
            b3lo_sb = cp.tile([K, NCASC], F32)
            nc.sync.dma_start(out=b3lo_sb[:], in_=b3lo[:])
            b3hi_sb = cp.tile([K, NCASC], F32)
            nc.sync.dma_start(out=b3hi_sb[:], in_=b3hi[:])
            zt = cp.tile([94, 64], BF16)
            nc.sync.dma_start(out=zt[:], in_=zz[:])

            # resident u (slab rows 1..11), fp32
            u_sb = up.tile([K, 11 * HW], F32)

            # ---- phase A: projection [u; c0] = G @ b  (bf16 hi+lo) ----
            for t in range(59):
                n0 = 512 * t
                nt = min(512, NSLAB - n0)
                bt = bp.tile([100, 6, 512], BF16, tag="bt")
                nc.gpsimd.dma_start(out=bt[:], in_=bcat[t])
                ps = pp.tile([111, 512], F32, tag="p1", bufs=2)
                for j in range(3):
                    nc.tensor.matmul(
                        ps[:, :nt], G_sb[:, j, :], bt[:, j, :nt],
                        start=(j == 0), stop=False,
                    )
                for j in range(3):
                    nc.tensor.matmul(
                        ps[:, :nt], Gl_sb[:, j, :], bt[:, j, :nt],
                        start=False, stop=False,
                    )
                for j in range(3):
                    nc.tensor.matmul(
                        ps[:, :nt], G_sb[:, j, :], bt[:, 3 + j, :nt],
                        start=False, stop=(j == 2),
                    )
                # u -> resident SBUF (only slab rows 1..11)
                lo = max(n0, HW)
                hi = min(n0 + nt, 12 * HW)
                if lo < hi:
                    nc.scalar.copy(
                        out=u_sb[:, lo - HW : hi - HW],
                        in_=ps[0:K, lo - n0 : hi - n0],
                    )
                # c0 -> DRAM (bf16)
                c0t = ep.tile([111, 512], BF16, tag="c0t")
                nc.vector.tensor_copy(out=c0t[64:111, :nt], in_=ps[64:111, :nt])
                nc.scalar.dma_start(
                    out=c_a[:, n0 : n0 + nt], in_=c0t[64:111, :nt]
                )
            nc.scalar.dma_start(out=c_a[:, NSLAB : NSLAB + 64], in_=zt[0:K, :])

            # ---- phase B: cascades ----
            cs = [c_a, c_b, c_a, c_b]
            for i in range(NCASC):
                cin = cs[i]
                cnext = cs[i + 1] if i < 3 else None
                start_c = W * i
                wl = W * (DFULL - 2 * i)
                c0c = W * (i + 1)
                wout = W * (46 - 2 * i)
                cbuf = cbp.tile([94, R * wl + 2 * PAD], BF16, tag="cbuf")
                nc.sync.dma_start(out=cbuf[:, 0:PAD], in_=zt[:, 0:PAD])

                def load_row(r):
                    base = ((r - i) % R) * wl + PAD
                    src = r * HW + start_c
                    nc.gpsimd.dma_start(
                        out=cbuf[0:K, base : base + wl + 1],
                        in_=cin[:, src : src + wl + 1],
                    )
                    nc.gpsimd.dma_start(
                        out=cbuf[K:94, base : base + wl + 1],
                        in_=cin[:, src + 1 : src + wl + 2],
                    )

                d0, d1 = i + 1, 12 - i
                for r in range(i, min(i + 3, 13 - i)):
                    load_row(r)
                for d in range(d0, d1):
                    if d + 2 < 13 - i:
                        load_row(d + 2)
                    cn_row = rp.tile(
                        [K, 2208], F32 if i == 3 else BF16,
                        tag="cnf" if i == 3 else "cnb",
                    )
                    cb = 0
                    for nt in _split_cols(wout):
                        def off(dz_row, delta):
                            s = ((d + dz_row - i) % R) * wl
                            return s + PAD + (c0c + cb - start_c) + delta

                        ps1 = pp.tile([HID, 512], F32, tag="p1", bufs=2)
                        first = True
                        for tz in range(3):
                            for ty in range(3):
                                o = off(tz - 1, (ty - 1) * W - 1)
                                nc.tensor.matmul(
                                    ps1[:, :nt],
                                    wp_sb[:, i, 3 * tz + ty, :],
                                    cbuf[0:94, o : o + nt],
                                    start=first, stop=False,
                                )
                                first = False
                        for tz in range(3):
                            for ty in range(3):
                                o = off(tz - 1, (ty - 1) * W + 1)
                                nc.tensor.matmul(
                                    ps1[:, :nt],
                                    ws_sb[:, i, 3 * tz + ty, :],
                                    cbuf[0:K, o : o + nt],
                                    start=False, stop=(tz == 2 and ty == 2),
                                )
                        r1 = ap_.tile([HID, 512], BF16, tag="r1")
                        nc.scalar.activation(
                            r1[:, :nt], ps1[:, :nt], AF.Relu, bias=b12_sb[:, i, 0:1]
                        )
                        ps2 = pp.tile([HID, 512], F32, tag="p2", bufs=2)
                        nc.tensor.matmul(
                            ps2[:, :nt], w2_sb[:, i, :], r1[:, :nt],
                            start=True, stop=True,
                        )
                        r2 = ap_.tile([HID, 512], BF16, tag="r2")
                        nc.scalar.activation(
                            r2[:, :nt], ps2[:, :nt], AF.Relu, bias=b12_sb[:, i, 1:2]
                        )
                        ps3a = pp.tile([K, 512], F32, tag="p3a", bufs=1)
                        nc.tensor.matmul(
                            ps3a[:, :nt], w3_sb[:, i, 0:K], r2[:, :nt],
                            start=True, stop=True,
                        )
                        ps3b = pp.tile([K, 512], F32, tag="p3b", bufs=1)
                        nc.tensor.matmul(
                            ps3b[:, :nt], w3_sb[:, i, K:94], r2[:, :nt],
                            start=True, stop=True,
                        )
                        sig = ap_.tile([K, 512], BF16, tag="sig")
                        nc.scalar.activation(
                            sig[:, :nt], ps3b[:, :nt], AF.Sigmoid,
                            bias=b3hi_sb[:, i : i + 1],
                        )
                        gt = ap_.tile([K, 512], BF16, tag="gt")
                        nc.vector.scalar_tensor_tensor(
                            out=gt[:, :nt],
                            in0=ps3a[:, :nt],
                            scalar=b3lo_sb[:, i : i + 1],
                            in1=sig[:, :nt],
                            op0=ALU.add,
                            op1=ALU.mult,
                        )
                        if i > 0:
                            nc.vector.tensor_add(
                                out=gt[:, :nt],
                                in0=gt[:, :nt],
                                in1=cbuf[0:K, off(0, 0) : off(0, 0) + nt],
                            )
                        ps4 = pp.tile([K, 512], F32, tag="p4", bufs=2)
                        nc.tensor.matmul(
                            ps4[:, :nt], s_sb[:], gt[:, :nt], start=True, stop=True
                        )
                        ucol = (d - 1) * HW + c0c + cb
                        nc.vector.tensor_add(
                            out=cn_row[:, cb : cb + nt],
                            in0=ps4[:, :nt],
                            in1=u_sb[:, ucol : ucol + nt],
                        )
                        cb += nt
                    if i < 3:
                        nc.gpsimd.dma_start(
                            out=cnext[:, d * HW + c0c : d * HW + c0c + wout],
                            in_=cn_row[:, :wout],
                        )
                        nc.scalar.dma_start(
                            out=cnext[
                                :, d * HW + c0c + wout : d * HW + c0c + wout + 49
                            ],
                            in_=cn_row[:, :49],
                        )
                    else:
                        nc.scalar.dma_start(
                            out=cout[:, d - 4, c0c : c0c + wout], in_=cn_row[:, :wout]
                        )

    nc.compile()
    return nc


_NC_CACHE = None


def _get_program():
    global _NC_CACHE
    if _NC_CACHE is None:
        _NC_CACHE = build_program()
    return _NC_CACHE


def prep_inputs(b, AQ, w1, b1, w2, b2, w3, b3, deep_reg):
    """Host-side: fold solves into matrices, repack weights, shard b."""
    AQ64 = np.asarray(AQ[0], np.float64)                     # (300, 47)
    dr = float(np.asarray(deep_reg))
    A = AQ64.T @ AQ64 + dr * np.eye(K)
    invA = np.linalg.inv(A)
    AQs = AQ64[:, IDX]
    A18 = AQs.T @ AQs + 0.01 * np.eye(len(IDX))
    invA18 = np.linalg.inv(A18)
    M0 = np.zeros((K, K))
    M0[np.ix_(IDX, IDX)] = invA18
    AQt = AQ64.T
    G = np.vstack([invA @ AQt, np.zeros((17, S)), M0 @ AQt])  # (111, 300)
    GT32 = np.ascontiguousarray(G.T, dtype=np.float32)        # (300, 111)
    GTh = GT32.astype(BF)
    GTl_h = (GT32 - GTh.astype(np.float32)).astype(BF)

    w1 = np.asarray(w1, np.float32)
    wp_h = np.empty((NCASC, 9, 94, HID), np.float32)
    ws_h = np.empty((NCASC, 9, K, HID), np.float32)
    for i in range(NCASC):
        for tz in range(3):
            for ty in range(3):
                t = 3 * tz + ty
                wp_h[i, t, 0:K] = w1[i, :, :, tz, ty, 0].T
                wp_h[i, t, K:94] = w1[i, :, :, tz, ty, 1].T
                ws_h[i, t] = w1[i, :, :, tz, ty, 2].T
    w2t_h = np.ascontiguousarray(
        np.transpose(np.asarray(w2, np.float32)[:, :, :, 0, 0, 0], (0, 2, 1))
    )
    w3t_h = np.ascontiguousarray(
        np.transpose(np.asarray(w3, np.float32)[:, :, :, 0, 0, 0], (0, 2, 1))
    )
    sT_h = np.ascontiguousarray((dr * invA).T, dtype=np.float32)
    b12_h = np.ascontiguousarray(
        np.stack([np.asarray(b1, np.float32).T, np.asarray(b2, np.float32).T], -1)
    )
    b3t = np.asarray(b3, np.float32).T
    shared = {
        "GT": GTh,
        "GTl": GTl_h,
        "wp": wp_h.astype(BF),
        "ws": ws_h.astype(BF),
        "w2t": w2t_h.astype(BF),
        "w3t": w3t_h.astype(BF),
        "sT": sT_h.astype(BF),
        "b12": b12_h,
        "b3lo": np.ascontiguousarray(b3t[0:K]),
        "b3hi": np.ascontiguousarray(b3t[K:94]),
        "zz": np.zeros((94, 64), np.float32).astype(BF),
    }
    v = np.asarray(b, np.float32)[0, :, :, :, :, 0].reshape(DFULL * HW, S)
    bT_full = np.ascontiguousarray(v.T)                      # (300, 110592)
    in_maps = []
    for k in range(NCORES):
        lo = 5 * k * HW
        m = dict(shared)
        sl = np.ascontiguousarray(bT_full[:, lo : lo + NSLAB])
        hi32 = sl.astype(BF).astype(np.float32)
        pads = np.zeros((300, 59 * 512 - NSLAB), np.float32)
        hi_p = np.concatenate([hi32, pads], 1)               # (300, 30208)
        lo_p = np.concatenate([sl - hi32, pads], 1)
        def tilemajor(a):
            return a.reshape(3, 100, 59, 512).transpose(2, 1, 0, 3)
        m["bcat"] = np.ascontiguousarray(
            np.concatenate([tilemajor(hi_p), tilemajor(lo_p)], 2)
        ).astype(BF)                                         # (59, 100, 6, 512)
        in_maps.append(m)
    return in_maps


def assemble(results):
    out = np.empty((1, 40, 40, 40, K, 1), np.float32)
    for k in range(NCORES):
        co = results[k]["cout"].reshape(K, OUTD, 48, 48)
        out[0, 5 * k : 5 * k + 5, :, :, :, 0] = np.transpose(
            co[:, :, 4:44, 4:44], (1, 2, 3, 0)
        )
    return out


def kernel(**inputs):
    nc = _get_program()
    in_maps = prep_inputs(**inputs)
    res = run_bass_kernel_spmd(nc, in_maps, core_ids=list(range(NCORES)))
    return assemble(res.results)


# revision 14
# speedup vs baseline: 1.0419x; 1.0419x over previous
"""Trainium2 Bass kernel for CSDNet (cascaded SH-fit + 3D conv refinement).

Math reformulation (host-side, exact up to fp assoc):
  AQ_Tb = AQ^T b per voxel;  A = AQ^T AQ + dr*I;  A18 = AQ_s^T AQ_s + 0.01 I
  c0 = M0 @ AQ_Tb  with  M0[IDX,IDX] = inv(A18)      (low-order init)
  u  = invA @ AQ_Tb                                   (data-consistency term)
  c_{i+1} = u + (dr*invA) @ g_i,   g_i = gate(cascade_i(c_i)) (+ c_i residual)
Both u and c0 come from one fused projection  [u; 0pad; c0] = G @ b_vox with
G = [invA; 0; M0] @ AQ^T  (111 x 300), so the big b read happens once.  The
projection runs as bf16 hi+lo split matmuls (~fp32 accuracy); u stays
resident in SBUF (fp32), c roundtrips DRAM as bf16.

Sharding: D axis split 8 ways; each core gets a 13-row overlapping slab
(5 output rows + 4 halo rows each side) so no inter-core communication is
needed.  Valid 3x3x3 convs are shifted-matmul accumulation over flattened
voxels; x-adjacent tap pairs share one K=94 matmul via a 1-shifted copy
of the activation rows (18 passes/conv instead of 27).  Compute is
restricted to the shrinking valid-H window per cascade.  Heavy DMA goes
through SWDGE (gpsimd), which spreads across all 16 SDMA engines; the
HWDGE rings (pinned to one engine here) carry only the cbuf row loads.
"""

import numpy as np
import ml_dtypes

import concourse.bacc as bacc
import concourse.bass as bass
import concourse.mybir as mybir
import concourse.tile as tile
from concourse.bass_utils import run_bass_kernel_spmd

F32 = mybir.dt.float32
BF16 = mybir.dt.bfloat16
BF = ml_dtypes.bfloat16

# problem constants (hardcoded per contract)
S = 300
K = 47
HID = 96
NCASC = 4
DFULL = 48
W = 48
HW = 48 * 48
NCORES = 8
SD = 13
NSLAB = SD * HW
OUTD = 5
R = 4                   # circular row slots
PAD = 49
NT_A = 512
IDX = np.r_[0:16, 45:47]


def _split_cols(width):
    out = []
    rem = width
    while rem > 0:
        if rem > 512:
            nt = 512 if rem - 512 >= 256 else rem - 256
        else:
            nt = rem
        out.append(nt)
        rem -= nt
    return out


def build_program():
    nc = bacc.Bacc(target_bir_lowering=False, trn_type="TRN2")

    bcat = nc.dram_tensor("bcat", [59, 100, 6, 512], BF16, kind="ExternalInput")
    GT = nc.dram_tensor("GT", [S, 111], BF16, kind="ExternalInput")
    GTl = nc.dram_tensor("GTl", [S, 111], BF16, kind="ExternalInput")
    wp = nc.dram_tensor("wp", [NCASC, 12, 94, HID], BF16, kind="ExternalInput")
    ws = nc.dram_tensor("ws", [NCASC, 3, K, HID], BF16, kind="ExternalInput")
    w2t = nc.dram_tensor("w2t", [NCASC, HID, HID], BF16, kind="ExternalInput")
    w3t = nc.dram_tensor("w3t", [NCASC, HID, 94], BF16, kind="ExternalInput")
    sT = nc.dram_tensor("sT", [K, K], BF16, kind="ExternalInput")
    b12 = nc.dram_tensor("b12", [HID, NCASC, 2], F32, kind="ExternalInput")
    b3lo = nc.dram_tensor("b3lo", [K, NCASC], F32, kind="ExternalInput")
    b3hi = nc.dram_tensor("b3hi", [K, NCASC], F32, kind="ExternalInput")
    zz = nc.dram_tensor("zz", [94, 64], BF16, kind="ExternalInput")

    c_a = nc.dram_tensor("c_a", [K, NSLAB + 64], BF16, kind="Internal")
    c_b = nc.dram_tensor("c_b", [K, NSLAB + 64], BF16, kind="Internal")
    cout = nc.dram_tensor("cout", [K, OUTD, HW], F32, kind="ExternalOutput")

    AF = mybir.ActivationFunctionType
    ALU = mybir.AluOpType

    with tile.TileContext(nc) as tc:
        with (
            tc.tile_pool(name="consts", bufs=1) as cp,
            tc.tile_pool(name="ubig", bufs=1) as up,
            tc.tile_pool(name="bstream", bufs=2) as bp,
            tc.tile_pool(name="evict", bufs=1) as ep,
            tc.tile_pool(name="cbufp", bufs=1) as cbp,
            tc.tile_pool(name="rows", bufs=1) as rp,
            tc.tile_pool(name="acts", bufs=2) as ap_,
            tc.tile_pool(name="pp", bufs=1, space="PSUM") as pp,
        ):
            # ---- constants ----
            G_sb = cp.tile([100, 3, 111], BF16)
            nc.sync.dma_start(
                out=G_sb[:], in_=GT[:].rearrange("(c p) m -> p c m", p=100)
            )
            Gl_sb = cp.tile([100, 3, 111], BF16)
            nc.sync.dma_start(
                out=Gl_sb[:], in_=GTl[:].rearrange("(c p) m -> p c m", p=100)
            )
            wp_sb = cp.tile([94, NCASC, 12, HID], BF16)
            nc.sync.dma_start(out=wp_sb[:], in_=wp[:].rearrange("i t p m -> p i t m"))
            ws_sb = cp.tile([K, NCASC, 3, HID], BF16)
            nc.sync.dma_start(out=ws_sb[:], in_=ws[:].rearrange("i t p m -> p i t m"))
            w2_sb = cp.tile([HID, NCASC, HID], BF16)
            nc.sync.dma_start(out=w2_sb[:], in_=w2t[:].rearrange("i p m -> p i m"))
            w3_sb = cp.tile([HID, NCASC, 94], BF16)
            nc.sync.dma_start(out=w3_sb[:], in_=w3t[:].rearrange("i p m -> p i m"))
            s_sb = cp.tile([K, K], BF16)
            nc.sync.dma_start(out=s_sb[:], in_=sT[:])
            b12_sb = cp.tile([HID, NCASC, 2], F32)
            nc.sync.dma_start(out=b12_sb[:], in_=b12[:])
            b3lo_sb = cp.tile([K, NCASC], F32)
            nc.sync.dma_start(out=b3lo_sb[:], in_=b3lo[:])
            b3hi_sb = cp.tile([K, NCASC], F32)
            nc.sync.dma_start(out=b3hi_sb[:], in_=b3hi[:])
            zt = cp.tile([94, 64], BF16)
            nc.sync.dma_start(out=zt[:], in_=zz[:])

            # resident u (slab rows 1..11), fp32
            u_sb = up.tile([K, 11 * HW], F32)

            # ---- phase A: projection [u; c0] = G @ b  (bf16 hi+lo) ----
            for t in range(59):
                n0 = 512 * t
                nt = min(512, NSLAB - n0)
                bt = bp.tile([100, 6, 512], BF16, tag="bt")
                nc.gpsimd.dma_start(out=bt[:], in_=bcat[t])
                ps = pp.tile([111, 512], F32, tag="p1", bufs=2)
                for j in range(3):
                    nc.tensor.matmul(
                        ps[:, :nt], G_sb[:, j, :], bt[:, j, :nt],
                        start=(j == 0), stop=False,
                    )
                for j in range(3):
                    nc.tensor.matmul(
                        ps[:, :nt], Gl_sb[:, j, :], bt[:, j, :nt],
                        start=False, stop=False,
                    )
                for j in range(3):
                    nc.tensor.matmul(
                        ps[:, :nt], G_sb[:, j, :], bt[:, 3 + j, :nt],
                        start=False, stop=(j == 2),
                    )
                # u -> resident SBUF (only slab rows 1..11)
                lo = max(n0, HW)
                hi = min(n0 + nt, 12 * HW)
                if lo < hi:
                    nc.scalar.copy(
                        out=u_sb[:, lo - HW : hi - HW],
                        in_=ps[0:K, lo - n0 : hi - n0],
                    )
                # c0 -> DRAM (bf16)
                c0t = ep.tile([111, 512], BF16, tag="c0t")
                nc.vector.tensor_copy(out=c0t[64:111, :nt], in_=ps[64:111, :nt])
                nc.scalar.dma_start(
                    out=c_a[:, n0 : n0 + nt], in_=c0t[64:111, :nt]
                )
            nc.scalar.dma_start(out=c_a[:, NSLAB : NSLAB + 64], in_=zt[0:K, :])

            # ---- phase B: cascades ----
            cs = [c_a, c_b, c_a, c_b]
            for i in range(NCASC):
                cin = cs[i]
                cnext = cs[i + 1] if i < 3 else None
                start_c = W * i
                wl = W * (DFULL - 2 * i)
                c0c = W * (i + 1)
                wout = W * (46 - 2 * i)
                cbuf = cbp.tile([94, R * wl + 2 * PAD], BF16, tag="cbuf")
                nc.sync.dma_start(out=cbuf[:, 0:PAD], in_=zt[:, 0:PAD])
                cbuf2 = cbp.tile([94, R * wl + 2 * PAD], BF16, tag="cbuf2")
                nc.sync.dma_start(out=cbuf2[:, 0:PAD], in_=zt[:, 0:PAD])

                def load_row(r):
                    base = ((r - i) % R) * wl + PAD
                    src = r * HW + start_c
                    nc.gpsimd.dma_start(
                        out=cbuf[0:K, base : base + wl + 1],
                        in_=cin[:, src : src + wl + 1],
                    )
                    nc.gpsimd.dma_start(
                        out=cbuf[K:94, base : base + wl + 1],
                        in_=cin[:, src + 1 : src + wl + 2],
                    )
                    nc.gpsimd.dma_start(
                        out=cbuf2[0:K, base : base + wl + 1],
                        in_=cin[:, src : src + wl + 1],
                    )
                    nc.gpsimd.dma_start(
                        out=cbuf2[K:94, base : base + wl + 1],
                        in_=cin[:, src + W : src + W + wl + 1],
                    )

                d0, d1 = i + 1, 12 - i
                for r in range(i, min(i + 3, 13 - i)):
                    load_row(r)
                for d in range(d0, d1):
                    if d + 2 < 13 - i:
                        load_row(d + 2)
                    cn_row = rp.tile(
                        [K, 2208], F32 if i == 3 else BF16,
                        tag="cnf" if i == 3 else "cnb",
                    )
                    cb = 0
                    for nt in _split_cols(wout):
                        def off(dz_row, delta):
                            s = ((d + dz_row - i) % R) * wl
                            return s + PAD + (c0c + cb - start_c) + delta

                        ps1 = pp.tile([HID, 512], F32, tag="p1", bufs=2)
                        first = True
                        for tz in range(3):
                            for k in range(3):
                                o = off(tz - 1, (k - 1) * W - 1)
                                nc.tensor.matmul(
                                    ps1[:, :nt],
                                    wp_sb[:, i, 4 * tz + k, :],
                                    cbuf[0:94, o : o + nt],
                                    start=first, stop=False,
                                )
                                first = False
                            o = off(tz - 1, -W + 1)
                            nc.tensor.matmul(
                                ps1[:, :nt],
                                wp_sb[:, i, 4 * tz + 3, :],
                                cbuf2[0:94, o : o + nt],
                                start=False, stop=False,
                            )
                        for tz in range(3):
                            o = off(tz - 1, W + 1)
                            nc.tensor.matmul(
                                ps1[:, :nt],
                                ws_sb[:, i, tz, :],
                                cbuf[0:K, o : o + nt],
                                start=False, stop=(tz == 2),
                            )
                        r1 = ap_.tile([HID, 512], BF16, tag="r1")
                        nc.scalar.activation(
                            r1[:, :nt], ps1[:, :nt], AF.Relu, bias=b12_sb[:, i, 0:1]
                        )
                        ps2 = pp.tile([HID, 512], F32, tag="p2", bufs=2)
                        nc.tensor.matmul(
                            ps2[:, :nt], w2_sb[:, i, :], r1[:, :nt],
                            start=True, stop=True,
                        )
                        r2 = ap_.tile([HID, 512], BF16, tag="r2")
                        nc.scalar.activation(
                            r2[:, :nt], ps2[:, :nt], AF.Relu, bias=b12_sb[:, i, 1:2]
                        )
                        ps3a = pp.tile([K, 512], F32, tag="p3a", bufs=1)
                        nc.tensor.matmul(
                            ps3a[:, :nt], w3_sb[:, i, 0:K], r2[:, :nt],
                            start=True, stop=True,
                        )
                        ps3b = pp.tile([K, 512], F32, tag="p3b", bufs=1)
                        nc.tensor.matmul(
                            ps3b[:, :nt], w3_sb[:, i, K:94], r2[:, :nt],
                            start=True, stop=True,
                        )
                        sig = ap_.tile([K, 512], BF16, tag="sig")
                        nc.scalar.activation(
                            sig[:, :nt], ps3b[:, :nt], AF.Sigmoid,
                            bias=b3hi_sb[:, i : i + 1],
                        )
                        gt = ap_.tile([K, 512], BF16, tag="gt")
                        nc.vector.scalar_tensor_tensor(
                            out=gt[:, :nt],
                            in0=ps3a[:, :nt],
                            scalar=b3lo_sb[:, i : i + 1],
                            in1=sig[:, :nt],
                            op0=ALU.add,
                            op1=ALU.mult,
                        )
                        if i > 0:
                            nc.vector.tensor_add(
                                out=gt[:, :nt],
                                in0=gt[:, :nt],
                                in1=cbuf[0:K, off(0, 0) : off(0, 0) + nt],
                            )
                        ps4 = pp.tile([K, 512], F32, tag="p4", bufs=2)
                        nc.tensor.matmul(
                            ps4[:, :nt], s_sb[:], gt[:, :nt], start=True, stop=True
                        )
                        ucol = (d - 1) * HW + c0c + cb
                        nc.vector.tensor_add(
                            out=cn_row[:, cb : cb + nt],
                            in0=ps4[:, :nt],
                            in1=u_sb[:, ucol : ucol + nt],
                        )
                        cb += nt
                    if i < 3:
                        nc.gpsimd.dma_start(
                            out=cnext[:, d * HW + c0c : d * HW + c0c + wout],
                            in_=cn_row[:, :wout],
                        )
                        nc.scalar.dma_start(
                            out=cnext[
                                :, d * HW + c0c + wout : d * HW + c0c + wout + 49
                            ],
                            in_=cn_row[:, :49],
                        )
                    else:
                        nc.scalar.dma_start(
                            out=cout[:, d - 4, c0c : c0c + wout], in_=cn_row[:, :wout]
                        )

    nc.compile()
    return nc


_NC_CACHE = None


def _get_program():
    global _NC_CACHE
    if _NC_CACHE is None:
        _NC_CACHE = build_program()
    return _NC_CACHE


def prep_inputs(b, AQ, w1, b1, w2, b2, w3, b3, deep_reg):
    """Host-side: fold solves into matrices, repack weights, shard b."""
    AQ64 = np.asarray(AQ[0], np.float64)                     # (300, 47)
    dr = float(np.asarray(deep_reg))
    A = AQ64.T @ AQ64 + dr * np.eye(K)
    invA = np.linalg.inv(A)
    AQs = AQ64[:, IDX]
    A18 = AQs.T @ AQs + 0.01 * np.eye(len(IDX))
    invA18 = np.linalg.inv(A18)
    M0 = np.zeros((K, K))
    M0[np.ix_(IDX, IDX)] = invA18
    AQt = AQ64.T
    G = np.vstack([invA @ AQt, np.zeros((17, S)), M0 @ AQt])  # (111, 300)
    GT32 = np.ascontiguousarray(G.T, dtype=np.float32)        # (300, 111)
    GTh = GT32.astype(BF)
    GTl_h = (GT32 - GTh.astype(np.float32)).astype(BF)

    w1 = np.asarray(w1, np.float32)
    wp_h = np.empty((NCASC, 12, 94, HID), np.float32)
    ws_h = np.empty((NCASC, 3, K, HID), np.float32)
    for i in range(NCASC):
        for tz in range(3):
            for k in range(3):    # x-pairs at ty=k: taps (tz,k,0)+(tz,k,1)
                wp_h[i, 4 * tz + k, 0:K] = w1[i, :, :, tz, k, 0].T
                wp_h[i, 4 * tz + k, K:94] = w1[i, :, :, tz, k, 1].T
            # y-pair: taps (tz,0,2)+(tz,1,2)
            wp_h[i, 4 * tz + 3, 0:K] = w1[i, :, :, tz, 0, 2].T
            wp_h[i, 4 * tz + 3, K:94] = w1[i, :, :, tz, 1, 2].T
            ws_h[i, tz] = w1[i, :, :, tz, 2, 2].T             # single (tz,2,2)
    w2t_h = np.ascontiguousarray(
        np.transpose(np.asarray(w2, np.float32)[:, :, :, 0, 0, 0], (0, 2, 1))
    )
    w3t_h = np.ascontiguousarray(
        np.transpose(np.asarray(w3, np.float32)[:, :, :, 0, 0, 0], (0, 2, 1))
    )
    sT_h = np.ascontiguousarray((dr * invA).T, dtype=np.float32)
    b12_h = np.ascontiguousarray(
        np.stack([np.asarray(b1, np.float32).T, np.asarray(b2, np.float32).T], -1)
    )
    b3t = np.asarray(b3, np.float32).T
    shared = {
        "GT": GTh,
        "GTl": GTl_h,
        "wp": wp_h.astype(BF),
        "ws": ws_h.astype(BF),
        "w2t": w2t_h.astype(BF),
        "w3t": w3t_h.astype(BF),
        "sT": sT_h.astype(BF),
        "b12": b12_h,
        "b3lo": np.ascontiguousarray(b3t[0:K]),
        "b3hi": np.ascontiguousarray(b3t[K:94]),
        "zz": np.zeros((94, 64), np.float32).astype(BF),
    }
    v = np.asarray(b, np.float32)[0, :, :, :, :, 0].reshape(DFULL * HW, S)
    bT_full = np.ascontiguousarray(v.T)                      # (300, 110592)
    in_maps = []
    for k in range(NCORES):
        lo = 5 * k * HW
        m = dict(shared)
        sl = np.ascontiguousarray(bT_full[:, lo : lo + NSLAB])
        hi32 = sl.astype(BF).astype(np.float32)
        pads = np.zeros((300, 59 * 512 - NSLAB), np.float32)
        hi_p = np.concatenate([hi32, pads], 1)               # (300, 30208)
        lo_p = np.concatenate([sl - hi32, pads], 1)
        def tilemajor(a):
            return a.reshape(3, 100, 59, 512).transpose(2, 1, 0, 3)
        m["bcat"] = np.ascontiguousarray(
            np.concatenate([tilemajor(hi_p), tilemajor(lo_p)], 2)
        ).astype(BF)                                         # (59, 100, 6, 512)
        in_maps.append(m)
    return in_maps


def assemble(results):
    out = np.empty((1, 40, 40, 40, K, 1), np.float32)
    for k in range(NCORES):
        co = results[k]["cout"].reshape(K, OUTD, 48, 48)
        out[0, 5 * k : 5 * k + 5, :, :, :, 0] = np.transpose(
            co[:, :, 4:44, 4:44], (1, 2, 3, 0)
        )
    return out


def kernel(**inputs):
    nc = _get_program()
    in_maps = prep_inputs(**inputs)
    res = run_bass_kernel_spmd(nc, in_maps, core_ids=list(range(NCORES)))
    return assemble(res.results)
